# revision 37
# baseline (speedup 1.0000x reference)
"""Trainium2 Bass kernel for nn_DeleteEdgeDecoder.

reference semantics (per batch b):
    feats[e] = [emb[i_e] | emb[j_e] | dist_e]          (513)
    h        = relu(feats @ W1 + b1)                   (E, 512)
    logits   = (h @ W2 + b2)[:, 0]  masked(-inf) + delete_bias

Sharding: pure data parallel, batch dim 128 -> 8 cores x 16.

Device dataflow per batch (edges-in-partitions orientation):
  - 32 indirect DMAs ([128,1] offsets - the stock DGE ucode consumes one
    offset per partition per call; wider offset APs silently misgather)
    fetch fp16 embedding rows edge-major: g[p, c, :] = emb[idx[c*128+p]]
  - per column: 4 PE transposes into a PSUM tile, then one DVE copy to
    fp16 ft[128f, kt, 2048e]; sides interleave per column so each
    gather's completion semaphore is consumed immediately (keeps the
    Pool SWDGE queue saturated) and so the last batch's compute
    pipelines with its own gathers
  - layer 1 per 128-edge tile: 4 accumulating matmuls with stationary
    ft k-tiles [128f x 128e] streaming W1' [128f x 512h], plus one K=2
    rank-2 matmul [dist;1] x [wd';b1'] into the same PSUM bank
    -> psum[e, h'] (one bank per e-tile)
  - layer 2 is folded into the relu: host scales W1 columns by |w2| and
    permutes them so positive-w2 columns come first (w2 relu(z) =
    sign(w2) relu(|w2| z)); two ACT relu calls per e-tile accumulate
    sum(relu) over the positive / negative column ranges via accum_out
  - logits = postbias + acc_pos - acc_neg: two small DVE ops per batch
  - one [128, 16] fp32 DMA out per batch; host unpacks e = c*128+p
"""

import os
from contextlib import ExitStack

import numpy as np
import concourse.bass as bass
import concourse.bacc as bacc
import concourse.mybir as mybir
import concourse.tile as tile
from concourse.bass_utils import run_bass_kernel_spmd

B, N, D, E = 128, 2000, 256, 2000
NCORES = 8
BL = B // NCORES          # batches per core
EP = 2048                 # edges padded to a multiple of 128
H = 512
EC = EP // 128            # 16 e-tiles of 128 edges
KT = (2 * D) // 128       # 4 k-tiles over [emb_i|emb_j]

F16 = mybir.dt.float16
F32 = mybir.dt.float32
I32 = mybir.dt.int32

_CACHE: dict = {}


def _build_nc(npos: int, bl: int = BL):
    """npos = number of h-columns with positive w2 (after host permutation)."""
    nc = bacc.Bacc(
        "TRN2", target_bir_lowering=False, debug=False, num_devices=NCORES
    )
    emb = nc.dram_tensor("emb", [bl * N, D], F16, kind="ExternalInput")
    idx = nc.dram_tensor("idx", [bl, 128, 2 * EC], I32, kind="ExternalInput")
    w1p = nc.dram_tensor("w1p", [128, KT, H], F16, kind="ExternalInput")
    wdb1 = nc.dram_tensor("wdb1", [2, H], F16, kind="ExternalInput")
    dist = nc.dram_tensor("dist", [bl, 2, EP], F16, kind="ExternalInput")
    ident = nc.dram_tensor("ident", [128, 128], F16, kind="ExternalInput")
    pbias = nc.dram_tensor("pbias", [bl, 128, EC], F32, kind="ExternalInput")
    out = nc.dram_tensor("out", [bl, 128, EC], F32, kind="ExternalOutput")

    with tile.TileContext(nc) as tc, ExitStack() as ctx:
        const = ctx.enter_context(tc.tile_pool(name="const", bufs=1))
        rawp = ctx.enter_context(tc.tile_pool(name="raw", bufs=2))
        ftp = ctx.enter_context(tc.tile_pool(name="ft", bufs=2))
        ipool = ctx.enter_context(tc.tile_pool(name="idx", bufs=2))
        spool = ctx.enter_context(tc.tile_pool(name="small", bufs=2))
        hpool = ctx.enter_context(tc.tile_pool(name="hrelu", bufs=3))
        apool = ctx.enter_context(tc.tile_pool(name="accs", bufs=2))
        opool = ctx.enter_context(tc.tile_pool(name="outt", bufs=2))
        psl1 = ctx.enter_context(tc.tile_pool(name="psl1", bufs=5, space="PSUM"))
        pstp = ctx.enter_context(tc.tile_pool(name="pstp", bufs=2, space="PSUM"))

        def load_phase(b):
            it = ipool.tile([128, 2 * EC], I32, tag="it")
            nc.sync.dma_start(it[:], idx.ap()[b])
            dist_t = spool.tile([2, EP], F16, tag="dist")
            nc.sync.dma_start(dist_t[:], dist.ap()[b])
            pb_t = spool.tile([128, EC], F32, tag="pb")
            nc.sync.dma_start(pb_t[:], pbias.ap()[b])

            # gather edge-major, sides interleaved per column so e-tile c
            # is computable right after its two gathers land; transposes
            # consume each gather's semaphore immediately and the psum->sbuf
            # copies ride the otherwise-idle DVE
            g0 = rawp.tile([128, EC, D], F16, tag="g0")
            g1 = rawp.tile([128, EC, D], F16, tag="g1")
            gs = [g0, g1]
            ft = ftp.tile([128, KT, EP], F16, tag="ft")
            for c in range(EC):
                pt = pstp.tile([128, KT, 128], F16, tag="tp")
                for side in range(2):
                    nc.gpsimd.indirect_dma_start(
                        out=gs[side][:, c, :],
                        out_offset=None,
                        in_=emb.ap(),
                        in_offset=bass.IndirectOffsetOnAxis(
                            ap=it[:, side * EC + c: side * EC + c + 1],
                            axis=0,
                        ),
                    )
                    for dk in range(2):
                        nc.tensor.transpose(
                            pt[:, side * 2 + dk, :],
                            gs[side][:, c, dk * 128:(dk + 1) * 128],
                            id_sb[:],
                        )
                nc.vector.tensor_copy(
                    ft[:, :, c * 128:(c + 1) * 128], pt[:]
                )
            return ft, dist_t, pb_t

        def compute_phase(b, tiles):
            ft, dist_t, pb_t = tiles
            apos = apool.tile([128, EC], F32, tag="apos")
            aneg = apool.tile([128, EC], F32, tag="aneg")
            for c in range(EC):
                ph = psl1.tile([128, H], F32, tag="l1")
                # dist[e]*wd'[h] + b1'[h] (K=2 rank-2) goes first: it only
                # needs dist, so it runs while the column's ft is still in
                # flight, shortening the per-column critical path
                nc.tensor.matmul(
                    ph[:],
                    dist_t[:, c * 128:(c + 1) * 128],
                    wdb1_sb[:],
                    start=True,
                    stop=False,
                )
                for kt in range(KT):
                    nc.tensor.matmul(
                        ph[:],
                        ft[:, kt, c * 128:(c + 1) * 128],
                        w1_sb[:, kt, :],
                        start=False,
                        stop=(kt == KT - 1),
                    )
                # relu + layer-2: accumulate sum(relu) over the +w2 and
                # -w2 column ranges (|w2| is folded into W1' on the host).
                # positive half on ACT, negative half on the idle DVE so the
                # per-tile epilogue is split across two engines (ACT alone
                # paces the drain otherwise)
                hs = hpool.tile([128, H], F16, tag="hs")
                if npos > 0:
                    nc.scalar.activation(
                        hs[:, :npos], ph[:, :npos],
                        mybir.ActivationFunctionType.Relu,
                        accum_out=apos[:, c:c + 1],
                    )
                if npos < H:
                    nc.vector.tensor_scalar_max(
                        hs[:, npos:], ph[:, npos:], 0.0
                    )
                    nc.vector.tensor_reduce(
                        aneg[:, c:c + 1], hs[:, npos:],
                        axis=mybir.AxisListType.X,
                        op=mybir.AluOpType.add,
                    )
            # logits = pb + apos - aneg  (two [128, EC] DVE ops per batch)
            lg = opool.tile([128, EC], F32, tag="lg")
            if npos == 0:
                nc.vector.memset(apos[:], 0.0)
            if npos == H:
                nc.vector.memset(aneg[:], 0.0)
            nc.vector.tensor_sub(lg[:], apos[:], aneg[:])
            nc.vector.tensor_add(lg[:], lg[:], pb_t[:])
            nc.sync.dma_start(out.ap()[b], lg[:])

        # const tiles exist up front, but their DMAs are emitted after
        # batch 0's index load so the first gathers start ~2us earlier
        w1_sb = const.tile([128, KT, H], F16)
        wdb1_sb = const.tile([2, H], F16)
        id_sb = const.tile([128, 128], F16)
        cur = load_phase(0)
        nc.sync.dma_start(w1_sb[:], w1p.ap())
        nc.sync.dma_start(wdb1_sb[:], wdb1.ap())
        nc.sync.dma_start(id_sb[:], ident.ap())

        for b in range(bl):
            compute_phase(b, cur)
            cur = load_phase(b + 1) if b + 1 < bl else None

    nc.compile()
    return nc


def _layer2_fold(W1, b1, W2):
    """Fold |w2| into the layer-1 columns; positives-first permutation.

    w2[h]*relu(z[h]) == sign(w2[h]) * relu(|w2[h]| * z[h]), so scaling
    every layer-1 output column by |w2| and splitting the relu-sum by
    sign computes layer 2 for free inside the activation.
    """
    w2 = np.asarray(W2).reshape(-1)
    pos = np.flatnonzero(w2 > 0)
    neg = np.flatnonzero(w2 <= 0)
    perm = np.concatenate([pos, neg])
    s = np.abs(w2[perm])
    W1f = W1[:, perm] * s[None, :]
    b1f = b1[perm] * s
    return W1f, b1f, int(len(pos))


def _prep_core_inputs(core, node_embeddings, locs, edge_list, delete_bias,
                      W1, b1, W2, b2, bl: int = BL):
    """Build the per-core input map (layout/dtype marshalling)."""
    b0 = core * bl
    emb16 = node_embeddings[b0 : b0 + bl].astype(np.float16).reshape(bl * N, D)

    el = edge_list[b0 : b0 + bl]  # (bl, E, 2) int
    iclip = np.maximum(el[..., 0], 0).astype(np.int64)
    jclip = np.maximum(el[..., 1], 0).astype(np.int64)
    # global row index into the per-core stacked embedding table
    base = (np.arange(bl, dtype=np.int64) * N)[:, None]
    gi = (iclip + base).astype(np.int32)
    gj = (jclip + base).astype(np.int32)

    def pack(idxv):  # (bl, E) -> (bl, 128, EC); tile[p, c] = idx[c*128+p]
        pad = np.zeros((bl, EP), dtype=np.int32)
        pad[:, :E] = idxv
        return pad.reshape(bl, EC, 128).transpose(0, 2, 1)

    idx = np.concatenate([pack(gi), pack(gj)], axis=2)  # (bl, 128, 2*EC)

    lc = locs[b0 : b0 + bl]
    bidx = np.arange(bl)[:, None]
    dvec = lc[bidx, iclip] - lc[bidx, jclip]
    distv = np.sqrt((dvec * dvec).sum(-1)).astype(np.float16)  # (bl, E)
    distones = np.zeros((bl, 2, EP), dtype=np.float16)
    distones[:, 0, :E] = distv
    distones[:, 1, :] = 1.0

    valid = (el[..., 0] >= 0) & (el[..., 1] >= 0)
    pb = np.where(valid, 0.0, -np.inf) + float(np.asarray(b2).reshape(-1)[0]) \
        + float(delete_bias)
    pbp = np.zeros((bl, EP), dtype=np.float32)
    pbp[:, :E] = pb
    pbp = pbp.reshape(bl, EC, 128).transpose(0, 2, 1)  # (bl, 128, EC)

    W1f, b1f, _ = _layer2_fold(W1, b1, W2)
    w1p = (
        W1f[: 2 * D].reshape(KT, 128, H).transpose(1, 0, 2).astype(np.float16)
    )
    wdb1 = np.stack([W1f[2 * D], b1f]).astype(np.float16)  # (2, H)

    return {
        "emb": emb16,
        "idx": np.ascontiguousarray(idx),
        "w1p": np.ascontiguousarray(w1p),
        "wdb1": np.ascontiguousarray(wdb1),
        "dist": distones,
        "pbias": np.ascontiguousarray(pbp),
        "ident": np.eye(128, dtype=np.float16),
    }


def kernel(node_embeddings, locs, edge_list, delete_bias, W1, b1, W2, b2):
    node_embeddings = np.asarray(node_embeddings, dtype=np.float32)
    locs = np.asarray(locs, dtype=np.float32)
    edge_list = np.asarray(edge_list)
    W1 = np.asarray(W1, dtype=np.float32)
    b1 = np.asarray(b1, dtype=np.float32)
    W2 = np.asarray(W2, dtype=np.float32)
    b2 = np.asarray(b2, dtype=np.float32)

    _, _, npos = _layer2_fold(W1, b1, W2)
    if _CACHE.get("npos") != npos:
        _CACHE["nc"] = _build_nc(npos)
        _CACHE["npos"] = npos
    nc = _CACHE["nc"]

    in_maps = [
        _prep_core_inputs(c, node_embeddings, locs, edge_list, delete_bias,
                          W1, b1, W2, b2)
        for c in range(NCORES)
    ]
    trace = os.environ.get("BASS_KERNEL_TRACE", "0") == "1"
    res = run_bass_kernel_spmd(nc, in_maps, list(range(NCORES)), trace=trace)
    _CACHE["last_result"] = res

    outs = []
    for c in range(NCORES):
        o = np.asarray(res.results[c]["out"], dtype=np.float32)
        # out[b, p, c] = logit(edge c*128+p)
        o = o.transpose(0, 2, 1).reshape(BL, EP)
        outs.append(o[:, :E])
    return np.concatenate(outs, axis=0)
